# revision 8
# baseline (speedup 1.0000x reference)
"""DeepseekV3 MoE layer on 8 Trainium2 NeuronCores.

Strategy (expert-parallel, per sharding hint):
- Each core owns 2 of the 16 routed experts. The host routes tokens to cores
  by top-k index lists (the all-to-all dispatch, done as input sharding): each
  core receives its experts' gathered tokens pre-transposed to [H, C] fp16.
- The device runs the SwiGLU expert MLP in fp16 (fp32 PSUM accumulation),
  computes the combine weights on-device (sigmoid gate + top-4
  normalization; the gate matmul rides the shared-expert gate/up matmul),
  scales expert outputs, and scatter-adds them into a partial-output buffer.
- The shared expert is sharded along its intermediate dim (128 of 1024 per
  core); its partial output initializes the partial-output buffer.
- The token dim is cut into NCH chunks; each chunk's ReduceScatter is
  triggered as soon as every scatter touching it has run, so almost all of
  the collective overlaps expert compute. The RS results are the kernel
  output directly (fp16); the host reassembles and casts to fp32.
"""

import os
import sys
import types

sys.path.insert(0, "/opt/trn_rl_repo")

# antenv.axon_hooks shim so trace=True works under axon (profiling only).
if "antenv.axon_hooks" not in sys.modules:
    _hook_holder = [None]
    _hooks_mod = types.ModuleType("antenv.axon_hooks")
    _hooks_mod.set_axon_ntff_profile_hook = lambda h: _hook_holder.__setitem__(0, h)
    _hooks_mod.get_axon_ntff_profile_hook = lambda: _hook_holder[0]
    sys.modules["antenv.axon_hooks"] = _hooks_mod
    try:
        from trn_agent_boot.trn_boot import _ntff_profile_via_ctypes

        _hook_holder[0] = _ntff_profile_via_ctypes("/opt/axon/libaxon_pjrt.so")
    except Exception:
        pass

import numpy as np

import concourse.bass as bass
import concourse.mybir as mybir
from concourse import bacc
from concourse.tile import TileContext, add_dep_helper
from concourse.bass_utils import run_bass_kernel_spmd

N_CORES = 8
T, H, E, I = 2048, 1024, 16, 512
TOPK = 4
SIC = 128  # shared-expert intermediate slice per core (1024 / 8)
EPC = 2  # experts per core
OOB = 1 << 20
NCH = int(os.environ.get("KERNEL_NCH", "4"))  # reduce-scatter chunks (token dim)
CH = T // NCH  # tokens per chunk
RPC = CH // N_CORES  # output rows per chunk per core

F16 = mybir.dt.float16
F32 = mybir.dt.float32
I32 = mybir.dt.int32
AF = mybir.ActivationFunctionType

_nc_cache = {}
last_exec_time_ns = None


def _build(C_use, C_pad, touch_sets, scat_tiles):
    NCC = C_pad // 128
    NTI = T // 128
    SS = 2 * SIC  # 256; score columns live at [SS, SS+E)
    nc = bacc.Bacc(trn_type="TRN2", target_bir_lowering=False, num_devices=N_CORES)

    # ---- I/O ----
    xT16 = nc.dram_tensor("xT16", [H, T], F16, kind="ExternalInput")
    xgT16 = nc.dram_tensor("xgT16", [EPC, 128, H // 128, C_pad], F16, kind="ExternalInput")
    wg16 = nc.dram_tensor("wg16", [EPC, H, I], F16, kind="ExternalInput")
    wu16 = nc.dram_tensor("wu16", [EPC, H, I], F16, kind="ExternalInput")
    wd16 = nc.dram_tensor("wd16", [EPC, I, H], F16, kind="ExternalInput")
    # [sg_slice | su_slice | gate_w.T(permuted)] packed: [H, 2*SIC + E]
    sgsu16 = nc.dram_tensor("sgsu16", [H, 2 * SIC + E], F16, kind="ExternalInput")
    sd16 = nc.dram_tensor("sd16", [SIC, H], F16, kind="ExternalInput")
    gidx = nc.dram_tensor("gidx", [EPC, NCC, 128], I32, kind="ExternalInput")
    sidx = nc.dram_tensor("sidx", [EPC, NCC, 128], I32, kind="ExternalInput")
    ident = nc.dram_tensor("ident", [128, 128], F16, kind="ExternalInput")

    y_acc = nc.dram_tensor("y_acc", [T, H], F16)
    w2_d = nc.dram_tensor("w2_d", [T, EPC], F32)
    rs_b = nc.dram_tensor("rs_b", [NCH * RPC, H], F16)
    y_out = nc.dram_tensor("y_out", [NCH * RPC, H], F16, kind="ExternalOutput")

    with TileContext(nc) as tc:
        with (
            tc.tile_pool(name="res", bufs=1) as res,
            tc.tile_pool(name="sc", bufs=3) as scp,
            tc.tile_pool(name="yg", bufs=4) as ygp,
            tc.tile_pool(name="ps", bufs=2, space="PSUM") as ps,
        ):
            # ---- resident tiles ----
            xT_sb = res.tile([128, H // 128, T], F16, tag="xT")
            xgT_sb = res.tile([128, EPC, H // 128, C_pad], F16, tag="xgT")
            wg_sb = res.tile([128, EPC, H // 128, I], F16, tag="wg")
            wu_sb = res.tile([128, EPC, H // 128, I], F16, tag="wu")
            wd_sb = res.tile([128, EPC, I // 128, H], F16, tag="wd")
            sgsu_sb = res.tile([128, H // 128, SS + E], F16, tag="sgsu")
            sd_sb = res.tile([128, H], F16, tag="sd")
            gidx_sb = res.tile([128, EPC * NCC], I32, tag="gidx")
            sidx_sb = res.tile([128, EPC * NCC], I32, tag="sidx")
            id_sb = res.tile([128, 128], F16, tag="ident")
            p_sb = res.tile([128, EPC, I // 128, C_pad], F16, tag="p")
            w2_sb = res.tile([128, NTI, EPC], F32, tag="w2")
            wG_sb = res.tile([128, EPC * NCC, EPC], F32, tag="wG")
            sp_sb = res.tile([128, NTI, SIC], F16, tag="sp")
            spT_sb = res.tile([128, NTI, 128], F16, tag="spT")
            sc_sb = res.tile([128, NTI * E], F32, tag="scores")
            sig_sb = res.tile([128, NTI * E], F32, tag="sig")

            # ---- preload ----
            nc.scalar.dma_start(gidx_sb[:], gidx.ap().rearrange("e c p -> p (e c)"))
            nc.scalar.dma_start(sidx_sb[:], sidx.ap().rearrange("e c p -> p (e c)"))
            nc.scalar.dma_start(id_sb[:], ident[:])
            nc.scalar.dma_start(sgsu_sb[:], sgsu16.ap().rearrange("(o p) s -> p o s", p=128))
            nc.scalar.dma_start(sd_sb[:], sd16.ap())

            # activations on sync; expert weights stream on the gpsimd queue.
            # xT feeds the shared block from t=0; xgT is not needed until the
            # expert phase (~30us), so it loads after all xT quarters.
            TC = T // 4
            for q in range(4):
                nc.sync.dma_start(
                    xT_sb[:, :, q * TC:(q + 1) * TC],
                    xT16.ap()[:, q * TC:(q + 1) * TC].rearrange(
                        "(o p) t -> p o t", p=128))
            for e in range(EPC):
                nc.sync.dma_start(xgT_sb[:, e], xgT16.ap()[e])
            for e in range(EPC):
                nc.gpsimd.dma_start(
                    wg_sb[:, e], wg16.ap()[e].rearrange("(o p) i -> p o i", p=128))
                nc.gpsimd.dma_start(
                    wu_sb[:, e], wu16.ap()[e].rearrange("(o p) i -> p o i", p=128))
            for e in range(EPC):
                nc.gpsimd.dma_start(
                    wd_sb[:, e], wd16.ap()[e].rearrange("(o p) h -> p o h", p=128))

            # zero the pad columns of p (read by down-matmul lhsT chunks)
            if C_pad > C_use:
                nc.vector.memset(p_sb[:, :, :, C_use:C_pad], 0)

            # ---- shared expert, fused per token tile: gate/up (+ gate
            # scores riding along), silu*up, transpose, down, y_acc init ----
            dense_wr = []
            for ti in range(NTI):
                psu = ps.tile([128, SS + E], F32, tag="A")
                for ho in range(H // 128):
                    nc.tensor.matmul(
                        psu[:],
                        lhsT=xT_sb[:, ho, ti * 128:(ti + 1) * 128],
                        rhs=sgsu_sb[:, ho, :],
                        start=(ho == 0),
                        stop=(ho == H // 128 - 1),
                    )
                sg_t = scp.tile([128, SIC], F16, tag="sg_t")
                nc.scalar.activation(sg_t[:], psu[:, 0:SIC], AF.Silu)
                nc.vector.tensor_tensor(
                    out=sp_sb[:, ti, :], in0=sg_t[:], in1=psu[:, SIC:SS],
                    op=mybir.AluOpType.mult,
                )
                # stash raw scores; one batched sigmoid later (the scalar
                # engine reloads its table on every silu<->sigmoid switch)
                nc.vector.tensor_copy(sc_sb[:, ti * E:(ti + 1) * E], psu[:, SS:SS + E])
                tps = ps.tile([128, 128], F16, tag="B")
                nc.tensor.transpose(tps[:], sp_sb[:, ti, :], id_sb[:])
                nc.vector.tensor_copy(spT_sb[:, ti, :], tps[:])
                ysh = ps.tile([128, H], F32, tag="Y")
                for hf in range(2):
                    nc.tensor.matmul(
                        ysh[:, hf * 512:(hf + 1) * 512],
                        lhsT=spT_sb[:, ti, :],
                        rhs=sd_sb[:, hf * 512:(hf + 1) * 512],
                        start=True,
                        stop=True,
                    )
                ys = ygp.tile([128, H], F16, tag="ys", bufs=4)
                nc.scalar.activation(ys[:], ysh[:], AF.Copy)
                wr = nc.sync.dma_start(out=y_acc[ti * 128:(ti + 1) * 128, :], in_=ys[:])
                dense_wr.append(wr)

            # ---- combine weights: batched sigmoid + top-4 normalize ----
            nc.scalar.activation(sig_sb[:], sc_sb[:], AF.Sigmoid)
            for ti in range(NTI):
                m8 = scp.tile([128, 8], F32, tag="m8")
                nc.vector.max(out=m8[:], in_=sig_sb[:, ti * E:(ti + 1) * E])
                s4 = scp.tile([128, 1], F32, tag="s4")
                nc.vector.reduce_sum(out=s4[:], in_=m8[:, 0:TOPK], axis=mybir.AxisListType.X)
                r4 = scp.tile([128, 1], F32, tag="r4")
                nc.vector.reciprocal(r4[:], s4[:])
                nc.vector.tensor_scalar_mul(
                    w2_sb[:, ti, :], sig_sb[:, ti * E:ti * E + EPC], r4[:])
            w2_wr = nc.scalar.dma_start(
                w2_d.ap().rearrange("(t p) e -> p t e", p=128), w2_sb[:]
            )

            # gather the combine weights for each expert's token list
            for e in range(EPC):
                for cc in range(NCC):
                    j = e * NCC + cc
                    wg_g = nc.gpsimd.indirect_dma_start(
                        out=wG_sb[:, j, :],
                        out_offset=None,
                        in_=w2_d[:],
                        in_offset=bass.IndirectOffsetOnAxis(ap=gidx_sb[:, j:j + 1], axis=0),
                    )
                    add_dep_helper(wg_g.ins, w2_wr.ins, reason="gather w after w2 write")

            # ---- routed experts ----
            scat_insts = {}
            rs_insts = []
            last_scat = [None]

            def emit_gu(e, a, b):
                for it in range(I // 128):
                    pg_full = ps.tile([128, 512], F32, tag="A")
                    pg = pg_full[:, :b - a]
                    pu_full = ps.tile([128, 512], F32, tag="B")
                    pu = pu_full[:, :b - a]
                    for ho in range(H // 128):
                        nc.tensor.matmul(
                            pg[:],
                            lhsT=wg_sb[:, e, ho, it * 128:(it + 1) * 128],
                            rhs=xgT_sb[:, e, ho, a:b],
                            start=(ho == 0),
                            stop=(ho == H // 128 - 1),
                        )
                        nc.tensor.matmul(
                            pu[:],
                            lhsT=wu_sb[:, e, ho, it * 128:(it + 1) * 128],
                            rhs=xgT_sb[:, e, ho, a:b],
                            start=(ho == 0),
                            stop=(ho == H // 128 - 1),
                        )
                    sg2 = scp.tile([128, 512], F16, tag="sg2")
                    nc.scalar.activation(sg2[:, :b - a], pg[:], AF.Silu)
                    nc.vector.tensor_tensor(
                        out=p_sb[:, e, it, a:b], in0=sg2[:, :b - a], in1=pu[:],
                        op=mybir.AluOpType.mult,
                    )

            def emit_down(e, cc):
                j = e * NCC + cc
                py = ps.tile([128, H], F32, tag="Y")
                for it in range(I // 128):
                    for hf in range(2):
                        nc.tensor.matmul(
                            py[:, hf * 512:(hf + 1) * 512],
                            lhsT=p_sb[:, e, it, cc * 128:(cc + 1) * 128],
                            rhs=wd_sb[:, e, it, hf * 512:(hf + 1) * 512],
                            start=(it == 0),
                            stop=(it == I // 128 - 1),
                        )
                yg = ygp.tile([128, H], F16, tag="ygtile", bufs=6)
                nc.vector.tensor_scalar_mul(yg[:], py[:], wG_sb[:, j, e:e + 1])

                sc = nc.gpsimd.indirect_dma_start(
                    out=y_acc[:],
                    out_offset=bass.IndirectOffsetOnAxis(
                        ap=sidx_sb[:, j:j + 1], axis=0),
                    in_=yg[:],
                    in_offset=None,
                    bounds_check=T - 1,
                    oob_is_err=False,
                    compute_op=mybir.AluOpType.add,
                )
                # scatters RMW y_acc: order them after the dense init of the
                # tiles they touch, and serialize the scatter chain itself
                for t in scat_tiles[(e, cc)]:
                    add_dep_helper(sc.ins, dense_wr[t].ins, reason="scatter after dense init")
                if last_scat[0] is not None:
                    add_dep_helper(sc.ins, last_scat[0].ins, reason="serialize scatters")
                last_scat[0] = sc
                scat_insts[(e, cc)] = sc

            def emit_rs(k):
                cc_inst = nc.gpsimd.collective_compute(
                    "ReduceScatter",
                    mybir.AluOpType.add,
                    replica_groups=[list(range(N_CORES))],
                    ins=[y_acc.ap()[k * CH:(k + 1) * CH, :].opt()],
                    outs=[rs_b.ap()[k * RPC:(k + 1) * RPC, :].opt()],
                )
                for key in touch_sets[k]:
                    add_dep_helper(cc_inst.ins, scat_insts[key].ins, reason="rs after scatters")
                for t in range(k * (NTI // NCH), (k + 1) * (NTI // NCH)):
                    add_dep_helper(cc_inst.ins, dense_wr[t].ins, reason="rs after dense init")
                rs_insts.append(cc_inst)

            def try_emit_rs():
                while len(rs_insts) < NCH and all(
                        key in scat_insts for key in touch_sets[len(rs_insts)]):
                    emit_rs(len(rs_insts))

            # token segments (<=512 moving dim); downs emitted as soon as
            # their 128-col chunk is complete so scatters + RS chunks start
            # while later gate/up segments still run
            done_cc = 0
            s0 = 0
            while s0 < C_use:
                s1 = min(s0 + 512, C_use)
                emit_gu(0, s0, s1)
                emit_gu(1, s0, s1)
                while (done_cc + 1) * 128 <= s1 or (s1 == C_use and done_cc < NCC):
                    emit_down(0, done_cc)
                    emit_down(1, done_cc)
                    try_emit_rs()
                    done_cc += 1
                s0 = s1
            try_emit_rs()
            assert len(rs_insts) == NCH, f"only {len(rs_insts)} RS chunks emitted"

            # copy each RS result chunk to the output (also forces
            # end-of-kernel to wait for RS completion)
            for k in range(NCH):
                cp = nc.sync.dma_start(
                    y_out.ap()[k * RPC:(k + 1) * RPC, :],
                    rs_b.ap()[k * RPC:(k + 1) * RPC, :])
                add_dep_helper(cp.ins, rs_insts[k].ins, reason="copy rs chunk out")

    nc.compile()
    return nc


def _get_nc(C_use, C_pad, touch_sets, scat_tiles):
    key = (C_use, C_pad, touch_sets, tuple(sorted(scat_tiles.items())))
    if key not in _nc_cache:
        _nc_cache[key] = _build(C_use, C_pad, touch_sets, scat_tiles)
    return _nc_cache[key]


def kernel(hidden_states, gate_w, expert_gate, expert_up, expert_down,
           shared_gate, shared_up, shared_down):
    global last_exec_time_ns
    B, S, Hh = hidden_states.shape
    x = np.asarray(hidden_states, np.float32).reshape(-1, Hh)

    # ---- host-side routing: build per-expert token index lists (sharding) ----
    gw = np.asarray(gate_w, np.float32)
    logits = x @ gw.T
    scores = 1.0 / (1.0 + np.exp(-logits))
    # top-4 per token; stable sort matches jax.lax.top_k tie semantics
    order = np.argsort(-scores, axis=1, kind="stable")[:, :TOPK]
    sel = np.zeros((T, E), dtype=bool)
    sel[np.arange(T)[:, None], order] = True
    counts = sel.sum(0)
    C_use = int(max(64, -(-int(counts.max()) // 64) * 64))
    C_use = min(C_use, T)
    C_pad = -(-C_use // 128) * 128
    NCC = C_pad // 128

    gidx_all = np.zeros((E, C_pad), np.int32)
    sidx_all = np.full((E, C_pad), OOB, np.int32)
    for e in range(E):
        lst = np.nonzero(sel[:, e])[0].astype(np.int32)
        gidx_all[e, :len(lst)] = lst
        sidx_all[e, :len(lst)] = lst

    # ---- cast / pack per-core inputs (the all-to-all token dispatch) ----
    x16 = x.astype(np.float16)
    xT16 = np.ascontiguousarray(x16.T)
    eg = np.asarray(expert_gate, np.float32).astype(np.float16)
    eu = np.asarray(expert_up, np.float32).astype(np.float16)
    ed = np.asarray(expert_down, np.float32).astype(np.float16)
    sg = np.asarray(shared_gate, np.float32).astype(np.float16)
    su = np.asarray(shared_up, np.float32).astype(np.float16)
    sd = np.asarray(shared_down, np.float32).astype(np.float16)
    gwT = gw.T.astype(np.float16)  # [H, E]
    identity = np.eye(128, dtype=np.float16)

    in_maps = []
    for c in range(N_CORES):
        ex = [EPC * c + k for k in range(EPC)]
        perm = ex + [e for e in range(E) if e not in ex]
        # gathered + transposed tokens per local expert:
        # [EPC, 128p(h within chunk), H/128, C_pad] — contiguous C_pad runs
        xgT = np.stack([
            np.ascontiguousarray(
                x16[gidx_all[e]].T.reshape(H // 128, 128, C_pad).transpose(1, 0, 2))
            for e in ex
        ])
        in_maps.append({
            "xT16": xT16,
            "xgT16": xgT,
            "wg16": np.ascontiguousarray(eg[ex]),
            "wu16": np.ascontiguousarray(eu[ex]),
            "wd16": np.ascontiguousarray(ed[ex]),
            "sgsu16": np.ascontiguousarray(
                np.concatenate([sg[:, c * SIC:(c + 1) * SIC],
                                su[:, c * SIC:(c + 1) * SIC],
                                gwT[:, perm]], axis=1)),
            "sd16": np.ascontiguousarray(sd[c * SIC:(c + 1) * SIC, :]),
            "gidx": np.ascontiguousarray(gidx_all[ex].reshape(EPC, NCC, 128)),
            "sidx": np.ascontiguousarray(sidx_all[ex].reshape(EPC, NCC, 128)),
            "ident": identity,
        })

    # which (local expert, chunk) scatters touch each RS token chunk, and
    # which dense-init tiles each scatter touches (union across cores so the
    # SPMD program is identical everywhere)
    touch = [set() for _ in range(NCH)]
    sctiles = {(k, i): set() for k in range(EPC) for i in range(NCC)}
    for c in range(N_CORES):
        for k, e in enumerate([EPC * c, EPC * c + 1]):
            for i in range(NCC):
                r = sidx_all[e, i * 128:(i + 1) * 128]
                r = r[r < OOB]
                if len(r):
                    lo, hi = int(r.min()), int(r.max())
                    for h in range(lo // CH, hi // CH + 1):
                        touch[h].add((k, i))
                    for t in range(lo // 128, hi // 128 + 1):
                        sctiles[(k, i)].add(t)
    touch_sets = tuple(frozenset(s) for s in touch)
    scat_tiles = {key: tuple(sorted(v)) for key, v in sctiles.items()}

    nc = _get_nc(C_use, C_pad, touch_sets, scat_tiles)
    trace = bool(int(os.environ.get("KERNEL_TRACE", "0")))
    res = run_bass_kernel_spmd(
        nc, in_maps, core_ids=list(range(N_CORES)), trace=trace
    )
    last_exec_time_ns = res.exec_time_ns

    # reassemble: RS chunk k gives core c rows [k*CH + c*RPC : +RPC] in its
    # y_out[k*RPC:(k+1)*RPC]
    out = np.empty((T, Hh), np.float32)
    for c in range(N_CORES):
        yo = res.results[c]["y_out"]
        for k in range(NCH):
            out[k * CH + c * RPC:k * CH + (c + 1) * RPC] = yo[k * RPC:(k + 1) * RPC]
    return out.reshape(B, S, Hh).astype(np.float32)


# revision 12
# speedup vs baseline: 1.1133x; 1.1133x over previous
"""DeepseekV3 MoE layer on 8 Trainium2 NeuronCores.

Strategy (expert-parallel, per sharding hint):
- Each core owns 2 of the 16 routed experts. The host routes tokens to cores
  by top-k index lists (the all-to-all dispatch, done as input sharding): each
  core receives its experts' gathered tokens pre-transposed to [H, C] fp16.
- The device runs the SwiGLU expert MLP in fp16 (fp32 PSUM accumulation),
  computes the combine weights on-device (sigmoid gate + top-4
  normalization; the gate matmul rides the shared-expert gate/up matmul),
  scales expert outputs, and scatter-adds them into a partial-output buffer.
- The shared expert is sharded along its intermediate dim (128 of 1024 per
  core); its partial output initializes the partial-output buffer.
- The token dim is cut into NCH chunks; each chunk's ReduceScatter is
  triggered as soon as every scatter touching it has run, so almost all of
  the collective overlaps expert compute. The RS results are the kernel
  output directly (fp16); the host reassembles and casts to fp32.
"""

import os
import sys
import types

sys.path.insert(0, "/opt/trn_rl_repo")

# antenv.axon_hooks shim so trace=True works under axon (profiling only).
if "antenv.axon_hooks" not in sys.modules:
    _hook_holder = [None]
    _hooks_mod = types.ModuleType("antenv.axon_hooks")
    _hooks_mod.set_axon_ntff_profile_hook = lambda h: _hook_holder.__setitem__(0, h)
    _hooks_mod.get_axon_ntff_profile_hook = lambda: _hook_holder[0]
    sys.modules["antenv.axon_hooks"] = _hooks_mod
    try:
        from trn_agent_boot.trn_boot import _ntff_profile_via_ctypes

        _hook_holder[0] = _ntff_profile_via_ctypes("/opt/axon/libaxon_pjrt.so")
    except Exception:
        pass

import numpy as np

import concourse.bass as bass
import concourse.mybir as mybir
from concourse import bacc
from concourse.tile import TileContext, add_dep_helper
from concourse.bass_utils import run_bass_kernel_spmd

N_CORES = 8
T, H, E, I = 2048, 1024, 16, 512
TOPK = 4
SIC = 128  # shared-expert intermediate slice per core (1024 / 8)
EPC = 2  # experts per core
OOB = 1 << 20
NCH = int(os.environ.get("KERNEL_NCH", "1"))  # reduce-scatter chunks (token dim)
NOLD = bool(int(os.environ.get("KERNEL_NOLD", "0")))  # skip dup weight loads
CH = T // NCH  # tokens per chunk
RPC = CH // N_CORES  # output rows per chunk per core

F16 = mybir.dt.float16
F32 = mybir.dt.float32
I32 = mybir.dt.int32
AF = mybir.ActivationFunctionType

_nc_cache = {}
last_exec_time_ns = None


def _build(C_use, C_pad, touch_sets, scat_tiles):
    NCC = C_pad // 128
    NTI = T // 128
    SS = 2 * SIC  # 256; score columns live at [SS, SS+E)
    nc = bacc.Bacc(trn_type="TRN2", target_bir_lowering=False, num_devices=N_CORES)

    # ---- I/O ----
    xT16 = nc.dram_tensor("xT16", [H, T], F16, kind="ExternalInput")
    xgT16 = nc.dram_tensor("xgT16", [EPC, 128, H // 128, C_pad], F16, kind="ExternalInput")
    wg16 = nc.dram_tensor("wg16", [EPC, H, I], F16, kind="ExternalInput")
    wu16 = nc.dram_tensor("wu16", [EPC, H, I], F16, kind="ExternalInput")
    wd16 = nc.dram_tensor("wd16", [EPC, I, H], F16, kind="ExternalInput")
    # [sg_slice | su_slice | gate_w.T(permuted)] packed: [H, 2*SIC + E]
    sgsu16 = nc.dram_tensor("sgsu16", [H, 2 * SIC + E], F16, kind="ExternalInput")
    sd16 = nc.dram_tensor("sd16", [SIC, H], F16, kind="ExternalInput")
    gidx = nc.dram_tensor("gidx", [EPC, NCC, 128], I32, kind="ExternalInput")
    sidx = nc.dram_tensor("sidx", [EPC, NCC, 128], I32, kind="ExternalInput")
    ident = nc.dram_tensor("ident", [128, 128], F16, kind="ExternalInput")

    y_acc = nc.dram_tensor("y_acc", [T, H], F16)
    w2_d = nc.dram_tensor("w2_d", [T, EPC], F32)
    rs_b = nc.dram_tensor("rs_b", [NCH * RPC, H], F16)
    y_out = nc.dram_tensor("y_out", [NCH * RPC, H], F16, kind="ExternalOutput")

    with TileContext(nc) as tc:
        with (
            tc.tile_pool(name="res", bufs=1) as res,
            tc.tile_pool(name="sc", bufs=3) as scp,
            tc.tile_pool(name="yg", bufs=4) as ygp,
            tc.tile_pool(name="ps", bufs=2, space="PSUM") as ps,
        ):
            # ---- resident tiles ----
            xT_sb = res.tile([128, H // 128, T], F16, tag="xT")
            xgT_sb = res.tile([128, EPC, H // 128, C_pad], F16, tag="xgT")
            wg_sb = res.tile([128, EPC, H // 128, I], F16, tag="wg")
            wu_sb = res.tile([128, EPC, H // 128, I], F16, tag="wu")
            wd_sb = res.tile([128, EPC, I // 128, H], F16, tag="wd")
            sgsu_sb = res.tile([128, H // 128, SS + E], F16, tag="sgsu")
            sd_sb = res.tile([128, H], F16, tag="sd")
            gidx_sb = res.tile([128, EPC * NCC], I32, tag="gidx")
            sidx_sb = res.tile([128, EPC * NCC], I32, tag="sidx")
            id_sb = res.tile([128, 128], F16, tag="ident")
            p_sb = res.tile([128, EPC, I // 128, C_pad], F16, tag="p")
            w2_sb = res.tile([128, NTI, EPC], F32, tag="w2")
            wG_sb = res.tile([128, EPC * NCC, EPC], F32, tag="wG")
            sp_sb = res.tile([128, NTI, SIC], F16, tag="sp")
            spT_sb = res.tile([128, NTI, 128], F16, tag="spT")
            sc_sb = res.tile([128, NTI * E], F32, tag="scores")
            sig_sb = res.tile([128, NTI * E], F32, tag="sig")

            # ---- preload ----
            # sgsu gates the very first matmul: it goes first on scalar.
            sgsu_wr = nc.scalar.dma_start(
                sgsu_sb[:], sgsu16.ap().rearrange("(o p) s -> p o s", p=128))
            nc.scalar.dma_start(id_sb[:], ident[:])
            nc.scalar.dma_start(sd_sb[:], sd16.ap())
            nc.scalar.dma_start(gidx_sb[:], gidx.ap().rearrange("e c p -> p (e c)"))
            nc.scalar.dma_start(sidx_sb[:], sidx.ap().rearrange("e c p -> p (e c)"))

            # activations on sync; expert weights stream on the gpsimd queue.
            # xT feeds the shared block from t=0; xgT is not needed until the
            # expert phase (~30us), so it loads after all xT quarters.
            TC = T // 4
            xt_wr = []
            for q in range(4):
                xt_wr.append(nc.sync.dma_start(
                    xT_sb[:, :, q * TC:(q + 1) * TC],
                    xT16.ap()[:, q * TC:(q + 1) * TC].rearrange(
                        "(o p) t -> p o t", p=128)))
            for e in range(EPC):
                nc.sync.dma_start(xgT_sb[:, e], xgT16.ap()[e])
            # weights are not needed until ~30us; let the first xT quarter
            # and sgsu have the HBM to themselves
            for e in range(EPC):
                w1 = nc.gpsimd.dma_start(
                    wg_sb[:, e], wg16.ap()[e].rearrange("(o p) i -> p o i", p=128))
                w2 = nc.gpsimd.dma_start(
                    wu_sb[:, e], wu16.ap()[e].rearrange("(o p) i -> p o i", p=128))
                if e == 0:
                    add_dep_helper(w1.ins, xt_wr[0].ins, reason="priority: xT q0 first")
                    add_dep_helper(w1.ins, sgsu_wr.ins, reason="priority: sgsu first")
            for e in range(EPC):
                nc.gpsimd.dma_start(
                    wd_sb[:, e], wd16.ap()[e].rearrange("(o p) h -> p o h", p=128))

            # zero the pad columns of p (read by down-matmul lhsT chunks)
            if C_pad > C_use:
                nc.vector.memset(p_sb[:, :, :, C_use:C_pad], 0)

            # ---- shared expert, fused per token tile: gate/up (+ gate
            # scores riding along), silu*up, transpose, down, y_acc init ----
            dense_wr = []
            for ti in range(NTI):
                psu = ps.tile([128, SS + E], F32, tag="A")
                for ho in range(H // 128):
                    nc.tensor.matmul(
                        psu[:],
                        lhsT=xT_sb[:, ho, ti * 128:(ti + 1) * 128],
                        rhs=sgsu_sb[:, ho, :],
                        start=(ho == 0),
                        stop=(ho == H // 128 - 1),
                    )
                sg_t = scp.tile([128, SIC], F16, tag="sg_t")
                nc.scalar.activation(sg_t[:], psu[:, 0:SIC], AF.Silu)
                nc.vector.tensor_tensor(
                    out=sp_sb[:, ti, :], in0=sg_t[:], in1=psu[:, SIC:SS],
                    op=mybir.AluOpType.mult,
                )
                # stash raw scores; one batched sigmoid later (the scalar
                # engine reloads its table on every silu<->sigmoid switch)
                nc.vector.tensor_copy(sc_sb[:, ti * E:(ti + 1) * E], psu[:, SS:SS + E])
                tps = ps.tile([128, 128], F16, tag="B")
                nc.tensor.transpose(tps[:], sp_sb[:, ti, :], id_sb[:])
                nc.vector.tensor_copy(spT_sb[:, ti, :], tps[:])
                ysh = ps.tile([128, H], F32, tag="Y")
                for hf in range(2):
                    mm = nc.tensor.matmul(
                        ysh[:, hf * 512:(hf + 1) * 512],
                        lhsT=spT_sb[:, ti, :],
                        rhs=sd_sb[:, hf * 512:(hf + 1) * 512],
                        start=True,
                        stop=True,
                    )
                    if NOLD and hf == 1:
                        mm.ins.ldweights = False
                ys = ygp.tile([128, H], F16, tag="ys", bufs=4)
                nc.scalar.activation(ys[:], ysh[:], AF.Copy)
                wr = nc.sync.dma_start(out=y_acc[ti * 128:(ti + 1) * 128, :], in_=ys[:])
                dense_wr.append(wr)

            # ---- combine weights: batched sigmoid + top-4 normalize ----
            nc.scalar.activation(sig_sb[:], sc_sb[:], AF.Sigmoid)
            for ti in range(NTI):
                m8 = scp.tile([128, 8], F32, tag="m8")
                nc.vector.max(out=m8[:], in_=sig_sb[:, ti * E:(ti + 1) * E])
                s4 = scp.tile([128, 1], F32, tag="s4")
                nc.vector.reduce_sum(out=s4[:], in_=m8[:, 0:TOPK], axis=mybir.AxisListType.X)
                r4 = scp.tile([128, 1], F32, tag="r4")
                nc.vector.reciprocal(r4[:], s4[:])
                nc.vector.tensor_scalar_mul(
                    w2_sb[:, ti, :], sig_sb[:, ti * E:ti * E + EPC], r4[:])
            w2_wr = nc.scalar.dma_start(
                w2_d.ap().rearrange("(t p) e -> p t e", p=128), w2_sb[:]
            )

            # gather the combine weights for each expert's token list
            for e in range(EPC):
                for cc in range(NCC):
                    j = e * NCC + cc
                    wg_g = nc.gpsimd.indirect_dma_start(
                        out=wG_sb[:, j, :],
                        out_offset=None,
                        in_=w2_d[:],
                        in_offset=bass.IndirectOffsetOnAxis(ap=gidx_sb[:, j:j + 1], axis=0),
                    )
                    add_dep_helper(wg_g.ins, w2_wr.ins, reason="gather w after w2 write")

            # ---- routed experts ----
            scat_insts = {}
            rs_insts = []
            last_scat = [None]

            def emit_gu(e, a, b):
                for it in range(I // 128):
                    pg_full = ps.tile([128, 512], F32, tag="A")
                    pg = pg_full[:, :b - a]
                    pu_full = ps.tile([128, 512], F32, tag="B")
                    pu = pu_full[:, :b - a]
                    for ho in range(H // 128):
                        nc.tensor.matmul(
                            pg[:],
                            lhsT=wg_sb[:, e, ho, it * 128:(it + 1) * 128],
                            rhs=xgT_sb[:, e, ho, a:b],
                            start=(ho == 0),
                            stop=(ho == H // 128 - 1),
                        )
                        nc.tensor.matmul(
                            pu[:],
                            lhsT=wu_sb[:, e, ho, it * 128:(it + 1) * 128],
                            rhs=xgT_sb[:, e, ho, a:b],
                            start=(ho == 0),
                            stop=(ho == H // 128 - 1),
                        )
                    sg2 = scp.tile([128, 512], F16, tag="sg2")
                    nc.scalar.activation(sg2[:, :b - a], pg[:], AF.Silu)
                    nc.vector.tensor_tensor(
                        out=p_sb[:, e, it, a:b], in0=sg2[:, :b - a], in1=pu[:],
                        op=mybir.AluOpType.mult,
                    )

            def emit_down(e, cc):
                j = e * NCC + cc
                py = ps.tile([128, H], F32, tag="Y")
                for it in range(I // 128):
                    for hf in range(2):
                        mm = nc.tensor.matmul(
                            py[:, hf * 512:(hf + 1) * 512],
                            lhsT=p_sb[:, e, it, cc * 128:(cc + 1) * 128],
                            rhs=wd_sb[:, e, it, hf * 512:(hf + 1) * 512],
                            start=(it == 0),
                            stop=(it == I // 128 - 1),
                        )
                        if NOLD and hf == 1:
                            mm.ins.ldweights = False
                yg = ygp.tile([128, H], F16, tag="ygtile", bufs=6)
                nc.vector.tensor_scalar_mul(yg[:], py[:], wG_sb[:, j, e:e + 1])

                sc = nc.gpsimd.indirect_dma_start(
                    out=y_acc[:],
                    out_offset=bass.IndirectOffsetOnAxis(
                        ap=sidx_sb[:, j:j + 1], axis=0),
                    in_=yg[:],
                    in_offset=None,
                    bounds_check=T - 1,
                    oob_is_err=False,
                    compute_op=mybir.AluOpType.add,
                )
                # scatters RMW y_acc: order them after the dense init of the
                # tiles they touch, and serialize the scatter chain itself
                for t in scat_tiles[(e, cc)]:
                    add_dep_helper(sc.ins, dense_wr[t].ins, reason="scatter after dense init")
                if last_scat[0] is not None:
                    add_dep_helper(sc.ins, last_scat[0].ins, reason="serialize scatters")
                last_scat[0] = sc
                scat_insts[(e, cc)] = sc

            def emit_rs(k):
                cc_inst = nc.gpsimd.collective_compute(
                    "ReduceScatter",
                    mybir.AluOpType.add,
                    replica_groups=[list(range(N_CORES))],
                    ins=[y_acc.ap()[k * CH:(k + 1) * CH, :].opt()],
                    outs=[rs_b.ap()[k * RPC:(k + 1) * RPC, :].opt()],
                )
                for key in touch_sets[k]:
                    add_dep_helper(cc_inst.ins, scat_insts[key].ins, reason="rs after scatters")
                for t in range(k * (NTI // NCH), (k + 1) * (NTI // NCH)):
                    add_dep_helper(cc_inst.ins, dense_wr[t].ins, reason="rs after dense init")
                rs_insts.append(cc_inst)

            def try_emit_rs():
                while len(rs_insts) < NCH and all(
                        key in scat_insts for key in touch_sets[len(rs_insts)]):
                    emit_rs(len(rs_insts))

            # token segments (<=512 moving dim); downs emitted as soon as
            # their 128-col chunk is complete so scatters + RS chunks start
            # while later gate/up segments still run
            done_cc = 0
            s0 = 0
            while s0 < C_use:
                s1 = min(s0 + 512, C_use)
                emit_gu(0, s0, s1)
                emit_gu(1, s0, s1)
                while (done_cc + 1) * 128 <= s1 or (s1 == C_use and done_cc < NCC):
                    emit_down(0, done_cc)
                    emit_down(1, done_cc)
                    try_emit_rs()
                    done_cc += 1
                s0 = s1
            try_emit_rs()
            assert len(rs_insts) == NCH, f"only {len(rs_insts)} RS chunks emitted"

            # copy each RS result chunk to the output (also forces
            # end-of-kernel to wait for RS completion)
            for k in range(NCH):
                cp = nc.sync.dma_start(
                    y_out.ap()[k * RPC:(k + 1) * RPC, :],
                    rs_b.ap()[k * RPC:(k + 1) * RPC, :])
                add_dep_helper(cp.ins, rs_insts[k].ins, reason="copy rs chunk out")

    nc.compile()
    return nc


def _get_nc(C_use, C_pad, touch_sets, scat_tiles):
    key = (C_use, C_pad, touch_sets, tuple(sorted(scat_tiles.items())))
    if key not in _nc_cache:
        _nc_cache[key] = _build(C_use, C_pad, touch_sets, scat_tiles)
    return _nc_cache[key]


def kernel(hidden_states, gate_w, expert_gate, expert_up, expert_down,
           shared_gate, shared_up, shared_down):
    global last_exec_time_ns
    B, S, Hh = hidden_states.shape
    x = np.asarray(hidden_states, np.float32).reshape(-1, Hh)

    # ---- host-side routing: build per-expert token index lists (sharding) ----
    gw = np.asarray(gate_w, np.float32)
    logits = x @ gw.T
    scores = 1.0 / (1.0 + np.exp(-logits))
    # top-4 per token; stable sort matches jax.lax.top_k tie semantics
    order = np.argsort(-scores, axis=1, kind="stable")[:, :TOPK]
    sel = np.zeros((T, E), dtype=bool)
    sel[np.arange(T)[:, None], order] = True
    counts = sel.sum(0)
    C_use = int(max(64, -(-int(counts.max()) // 64) * 64))
    C_use = min(C_use, T)
    C_pad = -(-C_use // 128) * 128
    NCC = C_pad // 128

    gidx_all = np.zeros((E, C_pad), np.int32)
    sidx_all = np.full((E, C_pad), OOB, np.int32)
    for e in range(E):
        lst = np.nonzero(sel[:, e])[0].astype(np.int32)
        gidx_all[e, :len(lst)] = lst
        sidx_all[e, :len(lst)] = lst

    # ---- cast / pack per-core inputs (the all-to-all token dispatch) ----
    x16 = x.astype(np.float16)
    xT16 = np.ascontiguousarray(x16.T)
    eg = np.asarray(expert_gate, np.float32).astype(np.float16)
    eu = np.asarray(expert_up, np.float32).astype(np.float16)
    ed = np.asarray(expert_down, np.float32).astype(np.float16)
    sg = np.asarray(shared_gate, np.float32).astype(np.float16)
    su = np.asarray(shared_up, np.float32).astype(np.float16)
    sd = np.asarray(shared_down, np.float32).astype(np.float16)
    gwT = gw.T.astype(np.float16)  # [H, E]
    identity = np.eye(128, dtype=np.float16)

    in_maps = []
    for c in range(N_CORES):
        ex = [EPC * c + k for k in range(EPC)]
        perm = ex + [e for e in range(E) if e not in ex]
        # gathered + transposed tokens per local expert:
        # [EPC, 128p(h within chunk), H/128, C_pad] — contiguous C_pad runs
        xgT = np.stack([
            np.ascontiguousarray(
                x16[gidx_all[e]].T.reshape(H // 128, 128, C_pad).transpose(1, 0, 2))
            for e in ex
        ])
        in_maps.append({
            "xT16": xT16,
            "xgT16": xgT,
            "wg16": np.ascontiguousarray(eg[ex]),
            "wu16": np.ascontiguousarray(eu[ex]),
            "wd16": np.ascontiguousarray(ed[ex]),
            "sgsu16": np.ascontiguousarray(
                np.concatenate([sg[:, c * SIC:(c + 1) * SIC],
                                su[:, c * SIC:(c + 1) * SIC],
                                gwT[:, perm]], axis=1)),
            "sd16": np.ascontiguousarray(sd[c * SIC:(c + 1) * SIC, :]),
            "gidx": np.ascontiguousarray(gidx_all[ex].reshape(EPC, NCC, 128)),
            "sidx": np.ascontiguousarray(sidx_all[ex].reshape(EPC, NCC, 128)),
            "ident": identity,
        })

    # which (local expert, chunk) scatters touch each RS token chunk, and
    # which dense-init tiles each scatter touches (union across cores so the
    # SPMD program is identical everywhere)
    touch = [set() for _ in range(NCH)]
    sctiles = {(k, i): set() for k in range(EPC) for i in range(NCC)}
    for c in range(N_CORES):
        for k, e in enumerate([EPC * c, EPC * c + 1]):
            for i in range(NCC):
                r = sidx_all[e, i * 128:(i + 1) * 128]
                r = r[r < OOB]
                if len(r):
                    lo, hi = int(r.min()), int(r.max())
                    for h in range(lo // CH, hi // CH + 1):
                        touch[h].add((k, i))
                    for t in range(lo // 128, hi // 128 + 1):
                        sctiles[(k, i)].add(t)
    touch_sets = tuple(frozenset(s) for s in touch)
    scat_tiles = {key: tuple(sorted(v)) for key, v in sctiles.items()}

    nc = _get_nc(C_use, C_pad, touch_sets, scat_tiles)
    trace = bool(int(os.environ.get("KERNEL_TRACE", "0")))
    res = run_bass_kernel_spmd(
        nc, in_maps, core_ids=list(range(N_CORES)), trace=trace
    )
    last_exec_time_ns = res.exec_time_ns

    # reassemble: RS chunk k gives core c rows [k*CH + c*RPC : +RPC] in its
    # y_out[k*RPC:(k+1)*RPC]
    out = np.empty((T, Hh), np.float32)
    for c in range(N_CORES):
        yo = res.results[c]["y_out"]
        for k in range(NCH):
            out[k * CH + c * RPC:k * CH + (c + 1) * RPC] = yo[k * RPC:(k + 1) * RPC]
    return out.reshape(B, S, Hh).astype(np.float32)


# revision 17
# speedup vs baseline: 1.1832x; 1.0628x over previous
"""DeepseekV3 MoE layer on 8 Trainium2 NeuronCores.

Strategy (expert-parallel, per sharding hint):
- Each core owns 2 of the 16 routed experts. The host routes tokens to cores
  by top-k index lists (the all-to-all dispatch, done as input sharding): each
  core receives its experts' gathered tokens pre-transposed to [H, C] fp16,
  plus the per-token combine weights for its experts (host-side routing).
- The device runs the SwiGLU expert MLP in fp16 (fp32 PSUM accumulation),
  scales expert outputs by the combine weights, and scatter-adds them into a
  partial-output buffer in DRAM.
- The shared expert is sharded along its intermediate dim (128 of 1024 per
  core); its partial output initializes the partial-output buffer.
- One ReduceScatter sums the partials across cores; its result is copied to
  the output. The host reassembles the 8 slices and casts to fp32.

All inputs are host-packed partition-major so every DMA moves ~128 fat
descriptors (the DMA rings drain at descriptor-count parity, not bytes).
"""

import os
import sys
import types

sys.path.insert(0, "/opt/trn_rl_repo")

# antenv.axon_hooks shim so trace=True works under axon (profiling only).
if "antenv.axon_hooks" not in sys.modules:
    _hook_holder = [None]
    _hooks_mod = types.ModuleType("antenv.axon_hooks")
    _hooks_mod.set_axon_ntff_profile_hook = lambda h: _hook_holder.__setitem__(0, h)
    _hooks_mod.get_axon_ntff_profile_hook = lambda: _hook_holder[0]
    sys.modules["antenv.axon_hooks"] = _hooks_mod
    try:
        from trn_agent_boot.trn_boot import _ntff_profile_via_ctypes

        _hook_holder[0] = _ntff_profile_via_ctypes("/opt/axon/libaxon_pjrt.so")
    except Exception:
        pass

import numpy as np

import concourse.bass as bass
import concourse.mybir as mybir
from concourse import bacc
from concourse.tile import TileContext, add_dep_helper
from concourse.bass_utils import run_bass_kernel_spmd

N_CORES = 8
T, H, E, I = 2048, 1024, 16, 512
TOPK = 4
SIC = 128  # shared-expert intermediate slice per core (1024 / 8)
EPC = 2  # experts per core
OOB = 1 << 20
NOLD = bool(int(os.environ.get("KERNEL_NOLD", "0")))  # skip dup weight loads

F16 = mybir.dt.float16
F32 = mybir.dt.float32
I32 = mybir.dt.int32
AF = mybir.ActivationFunctionType

_nc_cache = {}
last_exec_time_ns = None


def _build(C_use, C_pad, edges, scat_tiles):
    NCC = C_pad // 128
    NTI = T // 128
    SS = 2 * SIC  # 256
    ROWS = T // N_CORES  # 256 output rows per core
    nc = bacc.Bacc(trn_type="TRN2", target_bir_lowering=False, num_devices=N_CORES)

    # ---- I/O (all packed partition-major on the host) ----
    xTq = nc.dram_tensor("xTq", [4, 128, H // 128, T // 4], F16, kind="ExternalInput")
    xgT16 = nc.dram_tensor("xgT16", [EPC, 128, H // 128, C_pad], F16, kind="ExternalInput")
    wg16 = nc.dram_tensor("wg16", [EPC, 128, H // 128, I], F16, kind="ExternalInput")
    wu16 = nc.dram_tensor("wu16", [EPC, 128, H // 128, I], F16, kind="ExternalInput")
    wd16 = nc.dram_tensor("wd16", [EPC, 128, I // 128, H], F16, kind="ExternalInput")
    sgsu16 = nc.dram_tensor("sgsu16", [128, H // 128, SS], F16, kind="ExternalInput")
    sd16 = nc.dram_tensor("sd16", [SIC, H], F16, kind="ExternalInput")
    sidx = nc.dram_tensor("sidx", [128, EPC * NCC], I32, kind="ExternalInput")
    wgt = nc.dram_tensor("wgt", [128, EPC * NCC, EPC], F32, kind="ExternalInput")
    ident = nc.dram_tensor("ident", [128, 128], F16, kind="ExternalInput")

    y_acc = nc.dram_tensor("y_acc", [T, H], F16)
    rs_b = nc.dram_tensor("rs_b", [ROWS, H], F16)
    y_out = nc.dram_tensor("y_out", [ROWS, H], F16, kind="ExternalOutput")

    with TileContext(nc) as tc:
        with (
            tc.tile_pool(name="res", bufs=1) as res,
            tc.tile_pool(name="sc", bufs=3) as scp,
            tc.tile_pool(name="yg", bufs=4) as ygp,
            tc.tile_pool(name="ps", bufs=2, space="PSUM") as ps,
        ):
            # ---- resident tiles ----
            # quarter-major so each xT quarter lands as one 8KB run/partition
            xT_sb = res.tile([128, 4, H // 128, T // 4], F16, tag="xT")
            xgT_sb = res.tile([128, EPC, H // 128, C_pad], F16, tag="xgT")
            wg_sb = res.tile([128, EPC, H // 128, I], F16, tag="wg")
            wu_sb = res.tile([128, EPC, H // 128, I], F16, tag="wu")
            wd_sb = res.tile([128, EPC, I // 128, H], F16, tag="wd")
            sgsu_sb = res.tile([128, H // 128, SS], F16, tag="sgsu")
            sd_sb = res.tile([128, H], F16, tag="sd")
            sidx_sb = res.tile([128, EPC * NCC], I32, tag="sidx")
            wG_sb = res.tile([128, EPC * NCC, EPC], F32, tag="wG")
            id_sb = res.tile([128, 128], F16, tag="ident")
            p_sb = res.tile([128, EPC, I // 128, C_pad], F16, tag="p")
            sp_sb = res.tile([128, NTI, SIC], F16, tag="sp")
            spT_sb = res.tile([128, NTI, 128], F16, tag="spT")

            # ---- preload (sgsu + xT q0 gate the first matmuls) ----
            sgsu_wr = nc.scalar.dma_start(sgsu_sb[:], sgsu16.ap())
            nc.scalar.dma_start(id_sb[:], ident[:])
            nc.scalar.dma_start(sd_sb[:], sd16.ap())
            nc.scalar.dma_start(sidx_sb[:], sidx.ap())
            nc.scalar.dma_start(wG_sb[:], wgt.ap())

            TC = T // 4
            xt_wr = []
            for q in range(4):
                xt_wr.append(nc.sync.dma_start(xT_sb[:, q], xTq.ap()[q]))
            for e in range(EPC):
                nc.sync.dma_start(xgT_sb[:, e], xgT16.ap()[e])
            # weights stream on gpsimd; not needed until the expert phase
            for e in range(EPC):
                w1 = nc.gpsimd.dma_start(wg_sb[:, e], wg16.ap()[e])
                nc.gpsimd.dma_start(wu_sb[:, e], wu16.ap()[e])
                if e == 0:
                    add_dep_helper(w1.ins, xt_wr[0].ins, reason="priority: xT q0 first")
                    add_dep_helper(w1.ins, sgsu_wr.ins, reason="priority: sgsu first")
            for e in range(EPC):
                nc.gpsimd.dma_start(wd_sb[:, e], wd16.ap()[e])

            # zero the pad columns of p (read by down-matmul lhsT chunks)
            if C_pad > C_use:
                nc.vector.memset(p_sb[:, :, :, C_use:C_pad], 0)

            # ---- shared expert, fused per token tile: gate/up, silu*up,
            # transpose, down, y_acc init ----
            dense_wr = []
            for ti in range(NTI):
                psu = ps.tile([128, SS], F32, tag="A")
                for ho in range(H // 128):
                    nc.tensor.matmul(
                        psu[:],
                        lhsT=xT_sb[:, ti // 4, ho, (ti % 4) * 128:(ti % 4 + 1) * 128],
                        rhs=sgsu_sb[:, ho, :],
                        start=(ho == 0),
                        stop=(ho == H // 128 - 1),
                    )
                sg_t = scp.tile([128, SIC], F16, tag="sg_t")
                nc.scalar.activation(sg_t[:], psu[:, 0:SIC], AF.Silu)
                nc.vector.tensor_tensor(
                    out=sp_sb[:, ti, :], in0=sg_t[:], in1=psu[:, SIC:SS],
                    op=mybir.AluOpType.mult,
                )
                tps = ps.tile([128, 128], F16, tag="B")
                nc.tensor.transpose(tps[:], sp_sb[:, ti, :], id_sb[:])
                nc.vector.tensor_copy(spT_sb[:, ti, :], tps[:])
                ysh = ps.tile([128, H], F32, tag="Y")
                for hf in range(2):
                    mm = nc.tensor.matmul(
                        ysh[:, hf * 512:(hf + 1) * 512],
                        lhsT=spT_sb[:, ti, :],
                        rhs=sd_sb[:, hf * 512:(hf + 1) * 512],
                        start=True,
                        stop=True,
                    )
                    if NOLD and hf == 1:
                        mm.ins.ldweights = False
                ys = ygp.tile([128, H], F16, tag="ys", bufs=4)
                nc.scalar.activation(ys[:], ysh[:], AF.Copy)
                wr = nc.sync.dma_start(out=y_acc[ti * 128:(ti + 1) * 128, :], in_=ys[:])
                dense_wr.append(wr)

            # ---- routed experts ----
            scat_insts = {}

            def emit_gu(e, a, b):
                for it in range(I // 128):
                    pg_full = ps.tile([128, 512], F32, tag="A")
                    pg = pg_full[:, :b - a]
                    pu_full = ps.tile([128, 512], F32, tag="B")
                    pu = pu_full[:, :b - a]
                    for ho in range(H // 128):
                        nc.tensor.matmul(
                            pg[:],
                            lhsT=wg_sb[:, e, ho, it * 128:(it + 1) * 128],
                            rhs=xgT_sb[:, e, ho, a:b],
                            start=(ho == 0),
                            stop=(ho == H // 128 - 1),
                        )
                        nc.tensor.matmul(
                            pu[:],
                            lhsT=wu_sb[:, e, ho, it * 128:(it + 1) * 128],
                            rhs=xgT_sb[:, e, ho, a:b],
                            start=(ho == 0),
                            stop=(ho == H // 128 - 1),
                        )
                    sg2 = scp.tile([128, 512], F16, tag="sg2")
                    nc.scalar.activation(sg2[:, :b - a], pg[:], AF.Silu)
                    nc.vector.tensor_tensor(
                        out=p_sb[:, e, it, a:b], in0=sg2[:, :b - a], in1=pu[:],
                        op=mybir.AluOpType.mult,
                    )

            def emit_down(e, cc):
                j = e * NCC + cc
                py = ps.tile([128, H], F32, tag="Y")
                for it in range(I // 128):
                    for hf in range(2):
                        mm = nc.tensor.matmul(
                            py[:, hf * 512:(hf + 1) * 512],
                            lhsT=p_sb[:, e, it, cc * 128:(cc + 1) * 128],
                            rhs=wd_sb[:, e, it, hf * 512:(hf + 1) * 512],
                            start=(it == 0),
                            stop=(it == I // 128 - 1),
                        )
                        if NOLD and hf == 1:
                            mm.ins.ldweights = False
                yg = ygp.tile([128, H], F16, tag="ygtile", bufs=6)
                nc.vector.tensor_scalar_mul(yg[:], py[:], wG_sb[:, j, e:e + 1])

                sc = nc.gpsimd.indirect_dma_start(
                    out=y_acc[:],
                    out_offset=bass.IndirectOffsetOnAxis(
                        ap=sidx_sb[:, j:j + 1], axis=0),
                    in_=yg[:],
                    in_offset=None,
                    bounds_check=T - 1,
                    oob_is_err=False,
                    compute_op=mybir.AluOpType.add,
                )
                # order RMW scatters after the dense init of the tiles they
                # touch, and serialize only colliding scatter pairs
                for t in scat_tiles[(e, cc)]:
                    add_dep_helper(sc.ins, dense_wr[t].ins, reason="scatter after dense init")
                for (oe, occ) in edges.get((e, cc), ()):
                    if (oe, occ) in scat_insts:
                        add_dep_helper(sc.ins, scat_insts[(oe, occ)].ins,
                                       reason="serialize colliding scatters")
                scat_insts[(e, cc)] = sc

            # token segments (<=512 moving dim); a 128-col chunk's down runs
            # as soon as its gate/up columns are complete
            done_cc = 0
            s0 = 0
            while s0 < C_use:
                s1 = min(s0 + 512, C_use)
                emit_gu(0, s0, s1)
                emit_gu(1, s0, s1)
                while (done_cc + 1) * 128 <= s1 or (s1 == C_use and done_cc < NCC):
                    emit_down(0, done_cc)
                    emit_down(1, done_cc)
                    done_cc += 1
                s0 = s1

            # ---- one ReduceScatter over the summed partials ----
            cc_inst = nc.gpsimd.collective_compute(
                "ReduceScatter",
                mybir.AluOpType.add,
                replica_groups=[list(range(N_CORES))],
                ins=[y_acc.ap().opt()],
                outs=[rs_b.ap().opt()],
            )
            for sc in scat_insts.values():
                add_dep_helper(cc_inst.ins, sc.ins, reason="rs after scatters")
            for wr in dense_wr:
                add_dep_helper(cc_inst.ins, wr.ins, reason="rs after dense init")

            cp = nc.sync.dma_start(y_out.ap(), rs_b.ap())
            add_dep_helper(cp.ins, cc_inst.ins, reason="copy rs out")

    nc.compile()
    return nc


def _get_nc(C_use, C_pad, edges, scat_tiles):
    key = (C_use, C_pad, NOLD,
           tuple(sorted((k, tuple(v)) for k, v in edges.items())),
           tuple(sorted(scat_tiles.items())))
    if key not in _nc_cache:
        _nc_cache[key] = _build(C_use, C_pad, edges, scat_tiles)
    return _nc_cache[key]


def kernel(hidden_states, gate_w, expert_gate, expert_up, expert_down,
           shared_gate, shared_up, shared_down):
    global last_exec_time_ns
    B, S, Hh = hidden_states.shape
    x = np.asarray(hidden_states, np.float32).reshape(-1, Hh)

    # ---- host-side routing: top-k expert choice + combine weights ----
    gw = np.asarray(gate_w, np.float32)
    logits = x @ gw.T
    scores = 1.0 / (1.0 + np.exp(-logits))
    # top-4 per token; stable sort matches jax.lax.top_k tie semantics
    order = np.argsort(-scores, axis=1, kind="stable")[:, :TOPK]
    topk_w = np.take_along_axis(scores, order, axis=1)
    topk_w = topk_w / (topk_w.sum(-1, keepdims=True) + 1e-20)
    w2 = np.zeros((T, E), np.float32)
    np.put_along_axis(w2, order, topk_w, axis=1)
    sel = w2 > 0
    counts = sel.sum(0)
    C_use = int(max(64, -(-int(counts.max()) // 64) * 64))
    C_use = min(C_use, T)
    C_pad = -(-C_use // 128) * 128
    NCC = C_pad // 128

    gidx_all = np.zeros((E, C_pad), np.int32)
    sidx_all = np.full((E, C_pad), OOB, np.int32)
    for e in range(E):
        lst = np.nonzero(sel[:, e])[0].astype(np.int32)
        gidx_all[e, :len(lst)] = lst
        sidx_all[e, :len(lst)] = lst

    # ---- cast / pack per-core inputs (the all-to-all token dispatch),
    # partition-major so each DMA is 128 fat descriptors ----
    x16 = x.astype(np.float16)
    # [4 quarters][128 part (h%128)][8 (h//128)][512 tokens]
    xTq = np.ascontiguousarray(
        x16.T.reshape(H // 128, 128, 4, T // 4).transpose(2, 1, 0, 3))
    eg = np.asarray(expert_gate, np.float32).astype(np.float16)
    eu = np.asarray(expert_up, np.float32).astype(np.float16)
    ed = np.asarray(expert_down, np.float32).astype(np.float16)
    sg = np.asarray(shared_gate, np.float32).astype(np.float16)
    su = np.asarray(shared_up, np.float32).astype(np.float16)
    sd = np.asarray(shared_down, np.float32).astype(np.float16)
    identity = np.eye(128, dtype=np.float16)

    def pack_w(w):  # [H, I] -> [128, H//128, I]
        return np.ascontiguousarray(w.reshape(-1, 128, w.shape[-1]).transpose(1, 0, 2))

    in_maps = []
    for c in range(N_CORES):
        ex = [EPC * c + k for k in range(EPC)]
        xgT = np.stack([
            np.ascontiguousarray(
                x16[gidx_all[e]].T.reshape(H // 128, 128, C_pad).transpose(1, 0, 2))
            for e in ex
        ])
        sgsu = np.concatenate(
            [sg[:, c * SIC:(c + 1) * SIC], su[:, c * SIC:(c + 1) * SIC]], axis=1)
        # combine weights in gathered layout: [128][e*NCC+cc][local e]
        wgt = np.zeros((128, EPC * NCC, EPC), np.float32)
        for k, e in enumerate(ex):
            wgt[:, k * NCC:(k + 1) * NCC, k] = \
                w2[gidx_all[e].reshape(NCC, 128), e].T
        in_maps.append({
            "xTq": xTq,
            "xgT16": xgT,
            "wg16": np.stack([pack_w(eg[e]) for e in ex]),
            "wu16": np.stack([pack_w(eu[e]) for e in ex]),
            "wd16": np.stack([pack_w(ed[e]) for e in ex]),
            "sgsu16": pack_w(sgsu),
            "sd16": np.ascontiguousarray(sd[c * SIC:(c + 1) * SIC, :]),
            "sidx": np.ascontiguousarray(
                sidx_all[ex].reshape(EPC * NCC, 128).T),
            "wgt": wgt,
            "ident": identity,
        })

    # scatter collision edges (union across cores so the SPMD program is
    # identical everywhere) and dense-init tiles each scatter touches
    edge_set = set()
    sctiles = {(k, i): set() for k in range(EPC) for i in range(NCC)}
    rng = {}
    for c in range(N_CORES):
        for k, e in enumerate([EPC * c, EPC * c + 1]):
            for i in range(NCC):
                r = sidx_all[e, i * 128:(i + 1) * 128]
                r = r[r < OOB]
                if len(r):
                    lo, hi = int(r.min()), int(r.max())
                    rng[(c, k, i)] = (lo, hi)
                    for t in range(lo // 128, hi // 128 + 1):
                        sctiles[(k, i)].add(t)
        for i in range(NCC):
            for j in range(NCC):
                a = rng.get((c, 0, i))
                b = rng.get((c, 1, j))
                if a and b and a[0] <= b[1] and b[0] <= a[1]:
                    # emission order: (0,cc), (1,cc) per cc ascending
                    if i <= j:
                        edge_set.add(((1, j), (0, i)))
                    else:
                        edge_set.add(((0, i), (1, j)))
    edges = {}
    for later, earlier in edge_set:
        edges.setdefault(later, []).append(earlier)
    edges = {k: tuple(sorted(v)) for k, v in edges.items()}
    scat_tiles = {key: tuple(sorted(v)) for key, v in sctiles.items()}

    nc = _get_nc(C_use, C_pad, edges, scat_tiles)
    trace = bool(int(os.environ.get("KERNEL_TRACE", "0")))
    res = run_bass_kernel_spmd(
        nc, in_maps, core_ids=list(range(N_CORES)), trace=trace
    )
    last_exec_time_ns = res.exec_time_ns

    # reassemble: the RS gives core c rows [c*256, (c+1)*256)
    ROWS = T // N_CORES
    out = np.empty((T, Hh), np.float32)
    for c in range(N_CORES):
        out[c * ROWS:(c + 1) * ROWS] = res.results[c]["y_out"]
    return out.reshape(B, S, Hh).astype(np.float32)


# revision 20
# speedup vs baseline: 1.2654x; 1.0695x over previous
"""DeepseekV3 MoE layer on 8 Trainium2 NeuronCores.

Strategy (expert-parallel, per sharding hint):
- Each core owns 2 of the 16 routed experts. The host routes tokens to cores
  by top-k index lists (the all-to-all dispatch, done as input sharding): each
  core receives its experts' gathered tokens pre-transposed to [H, C] fp16,
  plus the per-token combine weights for its experts (host-side routing).
- The device runs the SwiGLU expert MLP in fp16 (fp32 PSUM accumulation),
  scales expert outputs by the combine weights, and scatter-adds them into a
  partial-output buffer in DRAM.
- The shared expert is sharded along its intermediate dim (128 of 1024 per
  core); its partial output initializes the partial-output buffer.
- One ReduceScatter sums the partials across cores; its result is copied to
  the output. The host reassembles the 8 slices and casts to fp32.

All inputs are host-packed partition-major so every DMA moves ~128 fat
descriptors (the DMA rings drain at descriptor-count parity, not bytes).
"""

import os
import sys
import types

sys.path.insert(0, "/opt/trn_rl_repo")

# antenv.axon_hooks shim so trace=True works under axon (profiling only).
if "antenv.axon_hooks" not in sys.modules:
    _hook_holder = [None]
    _hooks_mod = types.ModuleType("antenv.axon_hooks")
    _hooks_mod.set_axon_ntff_profile_hook = lambda h: _hook_holder.__setitem__(0, h)
    _hooks_mod.get_axon_ntff_profile_hook = lambda: _hook_holder[0]
    sys.modules["antenv.axon_hooks"] = _hooks_mod
    try:
        from trn_agent_boot.trn_boot import _ntff_profile_via_ctypes

        _hook_holder[0] = _ntff_profile_via_ctypes("/opt/axon/libaxon_pjrt.so")
    except Exception:
        pass

import numpy as np

import concourse.bass as bass
import concourse.mybir as mybir
from concourse import bacc
from concourse.tile import TileContext, add_dep_helper
from concourse.bass_utils import run_bass_kernel_spmd

N_CORES = 8
T, H, E, I = 2048, 1024, 16, 512
TOPK = 4
SIC = 128  # shared-expert intermediate slice per core (1024 / 8)
EPC = 2  # experts per core
OOB = 1 << 20
NOLD = bool(int(os.environ.get("KERNEL_NOLD", "1")))  # skip dup weight loads

F16 = mybir.dt.float16
F32 = mybir.dt.float32
I32 = mybir.dt.int32
AF = mybir.ActivationFunctionType

_nc_cache = {}
last_exec_time_ns = None


def _build(C_use, C_pad, edges, scat_tiles):
    NCC = C_pad // 128
    NTI = T // 128
    SS = 2 * SIC  # 256
    ROWS = T // N_CORES  # 256 output rows per core
    nc = bacc.Bacc(trn_type="TRN2", target_bir_lowering=False, num_devices=N_CORES)

    # ---- I/O (all packed partition-major on the host) ----
    xTq = nc.dram_tensor("xTq", [4, 128, H // 128, T // 4], F16, kind="ExternalInput")
    xgT16 = nc.dram_tensor("xgT16", [EPC, 128, H // 128, C_pad], F16, kind="ExternalInput")
    wg16 = nc.dram_tensor("wg16", [EPC, 128, H // 128, I], F16, kind="ExternalInput")
    wu16 = nc.dram_tensor("wu16", [EPC, 128, H // 128, I], F16, kind="ExternalInput")
    wd16 = nc.dram_tensor("wd16", [EPC, 128, I // 128, H], F16, kind="ExternalInput")
    sgsu16 = nc.dram_tensor("sgsu16", [128, H // 128, SS], F16, kind="ExternalInput")
    sd16 = nc.dram_tensor("sd16", [SIC, H], F16, kind="ExternalInput")
    sidx = nc.dram_tensor("sidx", [128, EPC * NCC], I32, kind="ExternalInput")
    wgt = nc.dram_tensor("wgt", [128, EPC * NCC, EPC], F32, kind="ExternalInput")
    ident = nc.dram_tensor("ident", [128, 128], F16, kind="ExternalInput")

    y_acc = nc.dram_tensor("y_acc", [T, H], F16)
    rs_b = nc.dram_tensor("rs_b", [ROWS, H], F16)
    y_out = nc.dram_tensor("y_out", [ROWS, H], F16, kind="ExternalOutput")

    with TileContext(nc) as tc:
        with (
            tc.tile_pool(name="res", bufs=1) as res,
            tc.tile_pool(name="sc", bufs=3) as scp,
            tc.tile_pool(name="yg", bufs=4) as ygp,
            tc.tile_pool(name="ps", bufs=2, space="PSUM") as ps,
        ):
            # ---- resident tiles ----
            # quarter-major so each xT quarter lands as one 8KB run/partition
            xT_sb = res.tile([128, 4, H // 128, T // 4], F16, tag="xT")
            xgT_sb = res.tile([128, EPC, H // 128, C_pad], F16, tag="xgT")
            wg_sb = res.tile([128, EPC, H // 128, I], F16, tag="wg")
            wu_sb = res.tile([128, EPC, H // 128, I], F16, tag="wu")
            wd_sb = res.tile([128, EPC, I // 128, H], F16, tag="wd")
            sgsu_sb = res.tile([128, H // 128, SS], F16, tag="sgsu")
            sd_sb = res.tile([128, H], F16, tag="sd")
            sidx_sb = res.tile([128, EPC * NCC], I32, tag="sidx")
            wG_sb = res.tile([128, EPC * NCC, EPC], F32, tag="wG")
            id_sb = res.tile([128, 128], F16, tag="ident")
            p_sb = res.tile([128, EPC, I // 128, C_pad], F16, tag="p")
            sp_sb = res.tile([128, NTI, SIC], F16, tag="sp")
            spT_sb = res.tile([128, NTI, 128], F16, tag="spT")

            # ---- preload (sgsu + xT q0 gate the first matmuls) ----
            sgsu_wr = nc.scalar.dma_start(sgsu_sb[:], sgsu16.ap())
            nc.scalar.dma_start(id_sb[:], ident[:])
            nc.scalar.dma_start(sd_sb[:], sd16.ap())
            nc.scalar.dma_start(sidx_sb[:], sidx.ap())
            nc.scalar.dma_start(wG_sb[:], wgt.ap())

            TC = T // 4
            xt_wr = []
            for q in range(4):
                xt_wr.append(nc.sync.dma_start(xT_sb[:, q], xTq.ap()[q]))
            # gate all bulk traffic on the two loads that unblock compute
            # (the tile scheduler reorders DMAs, so every one needs the dep)
            bulk = []
            for e in range(EPC):
                bulk.append(nc.sync.dma_start(xgT_sb[:, e], xgT16.ap()[e]))
            for e in range(EPC):
                bulk.append(nc.gpsimd.dma_start(wg_sb[:, e], wg16.ap()[e]))
                bulk.append(nc.gpsimd.dma_start(wu_sb[:, e], wu16.ap()[e]))
            for e in range(EPC):
                bulk.append(nc.gpsimd.dma_start(wd_sb[:, e], wd16.ap()[e]))
            for b in bulk:
                add_dep_helper(b.ins, xt_wr[0].ins, reason="priority: xT q0 first")
                add_dep_helper(b.ins, sgsu_wr.ins, reason="priority: sgsu first")

            # zero the pad columns of p (read by down-matmul lhsT chunks)
            if C_pad > C_use:
                nc.vector.memset(p_sb[:, :, :, C_use:C_pad], 0)

            # ---- shared expert, fused per token tile: gate/up, silu*up,
            # transpose, down, y_acc init ----
            dense_wr = []
            for ti in range(NTI):
                psu = ps.tile([128, SS], F32, tag="A")
                for ho in range(H // 128):
                    nc.tensor.matmul(
                        psu[:],
                        lhsT=xT_sb[:, ti // 4, ho, (ti % 4) * 128:(ti % 4 + 1) * 128],
                        rhs=sgsu_sb[:, ho, :],
                        start=(ho == 0),
                        stop=(ho == H // 128 - 1),
                    )
                sg_t = scp.tile([128, SIC], F16, tag="sg_t")
                nc.scalar.activation(sg_t[:], psu[:, 0:SIC], AF.Silu)
                nc.vector.tensor_tensor(
                    out=sp_sb[:, ti, :], in0=sg_t[:], in1=psu[:, SIC:SS],
                    op=mybir.AluOpType.mult,
                )
                tps = ps.tile([128, 128], F16, tag="B")
                nc.tensor.transpose(tps[:], sp_sb[:, ti, :], id_sb[:])
                nc.vector.tensor_copy(spT_sb[:, ti, :], tps[:])
                ysh = ps.tile([128, H], F32, tag="Y")
                for hf in range(2):
                    mm = nc.tensor.matmul(
                        ysh[:, hf * 512:(hf + 1) * 512],
                        lhsT=spT_sb[:, ti, :],
                        rhs=sd_sb[:, hf * 512:(hf + 1) * 512],
                        start=True,
                        stop=True,
                    )
                    if NOLD and hf == 1:
                        mm.ins.ldweights = False
                ys = ygp.tile([128, H], F16, tag="ys", bufs=4)
                nc.scalar.activation(ys[:], ysh[:], AF.Copy)
                # alternate rings: each ring drains 2KB-descriptor writes
                # at ~87GB/s, so two rings halve the dense-init drain
                q = nc.sync if ti % 2 == 0 else nc.scalar
                wr = q.dma_start(out=y_acc[ti * 128:(ti + 1) * 128, :], in_=ys[:])
                dense_wr.append(wr)

            # ---- routed experts ----
            scat_insts = {}

            def emit_gu(e, a, b):
                for it in range(I // 128):
                    pg_full = ps.tile([128, 512], F32, tag="A")
                    pg = pg_full[:, :b - a]
                    pu_full = ps.tile([128, 512], F32, tag="B")
                    pu = pu_full[:, :b - a]
                    for ho in range(H // 128):
                        nc.tensor.matmul(
                            pg[:],
                            lhsT=wg_sb[:, e, ho, it * 128:(it + 1) * 128],
                            rhs=xgT_sb[:, e, ho, a:b],
                            start=(ho == 0),
                            stop=(ho == H // 128 - 1),
                        )
                        nc.tensor.matmul(
                            pu[:],
                            lhsT=wu_sb[:, e, ho, it * 128:(it + 1) * 128],
                            rhs=xgT_sb[:, e, ho, a:b],
                            start=(ho == 0),
                            stop=(ho == H // 128 - 1),
                        )
                    sg2 = scp.tile([128, 512], F16, tag="sg2")
                    nc.scalar.activation(sg2[:, :b - a], pg[:], AF.Silu)
                    nc.vector.tensor_tensor(
                        out=p_sb[:, e, it, a:b], in0=sg2[:, :b - a], in1=pu[:],
                        op=mybir.AluOpType.mult,
                    )

            def emit_down(e, cc):
                j = e * NCC + cc
                py = ps.tile([128, H], F32, tag="Y")
                for it in range(I // 128):
                    for hf in range(2):
                        mm = nc.tensor.matmul(
                            py[:, hf * 512:(hf + 1) * 512],
                            lhsT=p_sb[:, e, it, cc * 128:(cc + 1) * 128],
                            rhs=wd_sb[:, e, it, hf * 512:(hf + 1) * 512],
                            start=(it == 0),
                            stop=(it == I // 128 - 1),
                        )
                        if NOLD and hf == 1:
                            mm.ins.ldweights = False
                yg = ygp.tile([128, H], F16, tag="ygtile", bufs=6)
                nc.vector.tensor_scalar_mul(yg[:], py[:], wG_sb[:, j, e:e + 1])

                sc = nc.gpsimd.indirect_dma_start(
                    out=y_acc[:],
                    out_offset=bass.IndirectOffsetOnAxis(
                        ap=sidx_sb[:, j:j + 1], axis=0),
                    in_=yg[:],
                    in_offset=None,
                    bounds_check=T - 1,
                    oob_is_err=False,
                    compute_op=mybir.AluOpType.add,
                )
                # order RMW scatters after the dense init of the tiles they
                # touch, and serialize only colliding scatter pairs
                for t in scat_tiles[(e, cc)]:
                    add_dep_helper(sc.ins, dense_wr[t].ins, reason="scatter after dense init")
                for (oe, occ) in edges.get((e, cc), ()):
                    if (oe, occ) in scat_insts:
                        add_dep_helper(sc.ins, scat_insts[(oe, occ)].ins,
                                       reason="serialize colliding scatters")
                scat_insts[(e, cc)] = sc

            # token segments (<=512 moving dim); a 128-col chunk's down runs
            # as soon as its gate/up columns are complete
            done_cc = 0
            s0 = 0
            while s0 < C_use:
                s1 = min(s0 + 512, C_use)
                emit_gu(0, s0, s1)
                emit_gu(1, s0, s1)
                while (done_cc + 1) * 128 <= s1 or (s1 == C_use and done_cc < NCC):
                    emit_down(0, done_cc)
                    emit_down(1, done_cc)
                    done_cc += 1
                s0 = s1

            # ---- one ReduceScatter over the summed partials ----
            cc_inst = nc.gpsimd.collective_compute(
                "ReduceScatter",
                mybir.AluOpType.add,
                replica_groups=[list(range(N_CORES))],
                ins=[y_acc.ap().opt()],
                outs=[rs_b.ap().opt()],
            )
            for sc in scat_insts.values():
                add_dep_helper(cc_inst.ins, sc.ins, reason="rs after scatters")
            for wr in dense_wr:
                add_dep_helper(cc_inst.ins, wr.ins, reason="rs after dense init")

            cp = nc.sync.dma_start(y_out.ap(), rs_b.ap())
            add_dep_helper(cp.ins, cc_inst.ins, reason="copy rs out")

    nc.compile()
    return nc


def _get_nc(C_use, C_pad, edges, scat_tiles):
    key = (C_use, C_pad, NOLD,
           tuple(sorted((k, tuple(v)) for k, v in edges.items())),
           tuple(sorted(scat_tiles.items())))
    if key not in _nc_cache:
        _nc_cache[key] = _build(C_use, C_pad, edges, scat_tiles)
    return _nc_cache[key]


def kernel(hidden_states, gate_w, expert_gate, expert_up, expert_down,
           shared_gate, shared_up, shared_down):
    global last_exec_time_ns
    B, S, Hh = hidden_states.shape
    x = np.asarray(hidden_states, np.float32).reshape(-1, Hh)

    # ---- host-side routing: top-k expert choice + combine weights ----
    gw = np.asarray(gate_w, np.float32)
    logits = x @ gw.T
    scores = 1.0 / (1.0 + np.exp(-logits))
    # top-4 per token; stable sort matches jax.lax.top_k tie semantics
    order = np.argsort(-scores, axis=1, kind="stable")[:, :TOPK]
    topk_w = np.take_along_axis(scores, order, axis=1)
    topk_w = topk_w / (topk_w.sum(-1, keepdims=True) + 1e-20)
    w2 = np.zeros((T, E), np.float32)
    np.put_along_axis(w2, order, topk_w, axis=1)
    sel = w2 > 0
    counts = sel.sum(0)
    C_use = int(max(64, -(-int(counts.max()) // 64) * 64))
    C_use = min(C_use, T)
    C_pad = -(-C_use // 128) * 128
    NCC = C_pad // 128

    gidx_all = np.zeros((E, C_pad), np.int32)
    sidx_all = np.full((E, C_pad), OOB, np.int32)
    for e in range(E):
        lst = np.nonzero(sel[:, e])[0].astype(np.int32)
        gidx_all[e, :len(lst)] = lst
        sidx_all[e, :len(lst)] = lst

    # ---- cast / pack per-core inputs (the all-to-all token dispatch),
    # partition-major so each DMA is 128 fat descriptors ----
    x16 = x.astype(np.float16)
    # [4 quarters][128 part (h%128)][8 (h//128)][512 tokens]
    xTq = np.ascontiguousarray(
        x16.T.reshape(H // 128, 128, 4, T // 4).transpose(2, 1, 0, 3))
    eg = np.asarray(expert_gate, np.float32).astype(np.float16)
    eu = np.asarray(expert_up, np.float32).astype(np.float16)
    ed = np.asarray(expert_down, np.float32).astype(np.float16)
    sg = np.asarray(shared_gate, np.float32).astype(np.float16)
    su = np.asarray(shared_up, np.float32).astype(np.float16)
    sd = np.asarray(shared_down, np.float32).astype(np.float16)
    identity = np.eye(128, dtype=np.float16)

    def pack_w(w):  # [H, I] -> [128, H//128, I]
        return np.ascontiguousarray(w.reshape(-1, 128, w.shape[-1]).transpose(1, 0, 2))

    in_maps = []
    for c in range(N_CORES):
        ex = [EPC * c + k for k in range(EPC)]
        xgT = np.stack([
            np.ascontiguousarray(
                x16[gidx_all[e]].T.reshape(H // 128, 128, C_pad).transpose(1, 0, 2))
            for e in ex
        ])
        sgsu = np.concatenate(
            [sg[:, c * SIC:(c + 1) * SIC], su[:, c * SIC:(c + 1) * SIC]], axis=1)
        # combine weights in gathered layout: [128][e*NCC+cc][local e]
        wgt = np.zeros((128, EPC * NCC, EPC), np.float32)
        for k, e in enumerate(ex):
            wgt[:, k * NCC:(k + 1) * NCC, k] = \
                w2[gidx_all[e].reshape(NCC, 128), e].T
        in_maps.append({
            "xTq": xTq,
            "xgT16": xgT,
            "wg16": np.stack([pack_w(eg[e]) for e in ex]),
            "wu16": np.stack([pack_w(eu[e]) for e in ex]),
            "wd16": np.stack([pack_w(ed[e]) for e in ex]),
            "sgsu16": pack_w(sgsu),
            "sd16": np.ascontiguousarray(sd[c * SIC:(c + 1) * SIC, :]),
            "sidx": np.ascontiguousarray(
                sidx_all[ex].reshape(EPC * NCC, 128).T),
            "wgt": wgt,
            "ident": identity,
        })

    # scatter collision edges (union across cores so the SPMD program is
    # identical everywhere) and dense-init tiles each scatter touches
    edge_set = set()
    sctiles = {(k, i): set() for k in range(EPC) for i in range(NCC)}
    rng = {}
    for c in range(N_CORES):
        for k, e in enumerate([EPC * c, EPC * c + 1]):
            for i in range(NCC):
                r = sidx_all[e, i * 128:(i + 1) * 128]
                r = r[r < OOB]
                if len(r):
                    lo, hi = int(r.min()), int(r.max())
                    rng[(c, k, i)] = (lo, hi)
                    for t in range(lo // 128, hi // 128 + 1):
                        sctiles[(k, i)].add(t)
        for i in range(NCC):
            for j in range(NCC):
                a = rng.get((c, 0, i))
                b = rng.get((c, 1, j))
                if a and b and a[0] <= b[1] and b[0] <= a[1]:
                    # emission order: (0,cc), (1,cc) per cc ascending
                    if i <= j:
                        edge_set.add(((1, j), (0, i)))
                    else:
                        edge_set.add(((0, i), (1, j)))
    edges = {}
    for later, earlier in edge_set:
        edges.setdefault(later, []).append(earlier)
    edges = {k: tuple(sorted(v)) for k, v in edges.items()}
    scat_tiles = {key: tuple(sorted(v)) for key, v in sctiles.items()}

    nc = _get_nc(C_use, C_pad, edges, scat_tiles)
    trace = bool(int(os.environ.get("KERNEL_TRACE", "0")))
    res = run_bass_kernel_spmd(
        nc, in_maps, core_ids=list(range(N_CORES)), trace=trace
    )
    last_exec_time_ns = res.exec_time_ns

    # reassemble: the RS gives core c rows [c*256, (c+1)*256)
    ROWS = T // N_CORES
    out = np.empty((T, Hh), np.float32)
    for c in range(N_CORES):
        out[c * ROWS:(c + 1) * ROWS] = res.results[c]["y_out"]
    return out.reshape(B, S, Hh).astype(np.float32)
